# revision 12
# baseline (speedup 1.0000x reference)
"""Trainium2 Bass kernel for nn_AutoAttention_Layer (sparse_attention).

Math (from the reference):
    W    = softmax(mss_weight, axis=1)                      # (3,3)
    qsum = sum_j q[b,j,:]                                   # (B,D)
    ksum_s[b,d] = sum_{l < len[b]} k[b,l,s*D+d]             # (B,3,D)
    s[r,b,d]    = (sum_s W[r,s]*ksum_s[b,d]) * qsum[b,d]
    out[b,0,r*D+d] = softmax_d(s[r,b,:])
`v` is never used.

Strategy: pure data parallel over 8 NeuronCores (128 samples/core, batch on
SBUF partitions).  The masked sum over l is computed as 8-row block sums via
contiguous pairwise tensor_tensor add trees (1 output elem/cycle on VectorE;
a strided tensor_reduce measured only ~0.6 elem/cycle), then masked with
per-sample 0/1 block masks broadcast along d (stride-0 AP) and folded into a
running accumulator — all plain tensor_tensor, which uses only the DVE's
dedicated SBUF ports and never collides with GpSimd (the DVE<->GpSimd
shared-port lock measured as a 10x stall on per-partition-scalar ops).
All fold scratch lives inside the DMA'd k-chunk tiles themselves (writes
trail reads, so in-place pair-folds are safe), freeing SBUF for deep DMA
buffering.  GpSimd computes the q sum.  A partial-block correction uses 8
rows gathered at host-computed offsets (indices derive from kes_length on
the host; the row data itself is DMA'd from HBM).
"""

import numpy as np

try:
    import concourse.bass as bass
except ImportError:  # pragma: no cover - path fallback
    import sys

    sys.path.insert(0, "/opt/trn_rl_repo")
    import concourse.bass as bass

import concourse.bacc as bacc
import concourse.mybir as mybir
import concourse.tile as tile
from concourse.bass_utils import run_bass_kernel_spmd

F32 = mybir.dt.float32

NCORES = 8
B = 1024
BL = B // NCORES  # 128 samples per core = SBUF partitions
LQ = 64
LK = 200
D = 64
KD = 3 * D  # 192
G = 8  # rows per l-block
NB = LK // G  # 25 blocks
CHUNKS = [8, 48, 48, 48, 40, 8]  # tiny first (early start) and last (small tail)

_CACHE = {}


def _bcast_inner(ap, n):
    """View a (P, m) AP as (P, m, n) with stride-0 innermost broadcast."""
    return bass.AP(tensor=ap.tensor, offset=ap.offset, ap=[*ap.ap, [0, n]])


def _inplace_fold(eng, buf, rows, add):
    """Pairwise-fold (BL, rows, w) into (BL, rows//2, w) in the same tile.

    out row j = in rows 2j + 2j+1; writes trail reads (j <= 2j) so in-place
    is safe on the streaming engines.
    """
    nxt = rows // 2
    pairs = buf[:, 0 : 2 * nxt, :].rearrange("p (a two) d -> p a two d", two=2)
    eng.tensor_tensor(
        out=buf[:, 0:nxt, :], in0=pairs[:, :, 0, :], in1=pairs[:, :, 1, :], op=add
    )
    return nxt


def _build_module():
    nc = bacc.Bacc("TRN2", target_bir_lowering=False, debug=False)

    q_d = nc.dram_tensor("q", [BL, D, LQ], F32, kind="ExternalInput").ap()  # host-transposed (b, d, lq)
    k_d = nc.dram_tensor("k", [BL, LK, KD], F32, kind="ExternalInput").ap()
    kg_d = nc.dram_tensor("kg", [BL, G, KD], F32, kind="ExternalInput").ap()
    # meta = [w(9) | bm(25) | sm(8)] per partition
    meta_d = nc.dram_tensor("meta", [BL, 9 + NB + G], F32, kind="ExternalInput").ap()
    out_d = nc.dram_tensor("out", [BL, KD], F32, kind="ExternalOutput").ap()

    mult = mybir.AluOpType.mult
    add = mybir.AluOpType.add
    AX = mybir.AxisListType.X

    with tile.TileContext(nc) as tc:
        with (
            tc.tile_pool(name="singles", bufs=1) as singles,
            tc.tile_pool(name="kpool", bufs=4) as kpool,
            tc.tile_pool(name="spool", bufs=2) as spool,
            tc.tile_pool(name="small", bufs=2) as small,
        ):
            # --- DMAs: k chunks on the Sync HWDGE ring; everything small on
            # the Scalar HWDGE ring so it lands early without delaying k ---
            # DMA order: tiny chunk 0 first so DVE starts early, then the
            # small tensors inline (side rings starve: 0.8MB took 27us on the
            # ACT ring), q on the SWDGE queue, then the bulk k chunks.
            kcs = []
            l0 = 0
            kg_t = meta_t = q_t = None
            for ci, R in enumerate(CHUNKS):
                kc = kpool.tile([BL, R, KD], F32, tag="kc")
                nc.sync.dma_start(out=kc, in_=k_d[:, l0 : l0 + R, :])
                kcs.append((kc, R))
                l0 += R
                if ci == 0:
                    kg_t = singles.tile([BL, G, KD], F32)
                    nc.sync.dma_start(out=kg_t, in_=kg_d)
                    meta_t = singles.tile([BL, 9 + NB + G], F32)
                    nc.sync.dma_start(out=meta_t, in_=meta_d)
                    q_t = singles.tile([BL, D, LQ], F32)
                    nc.gpsimd.dma_start(out=q_t, in_=q_d)  # SWDGE queue

            w_t = meta_t[:, 0:9]
            bm_t = meta_t[:, 9 : 9 + NB]
            sm_t = meta_t[:, 9 + NB : 9 + NB + G]

            acc = singles.tile([BL, KD], F32)

            def chunk_chain(kc, R, jg, seed):
                # pairwise tree: L1 out of the kc tile (frees its DMA slot),
                # deeper levels in place in the scratch; then one masked
                # scalar_tensor_tensor accumulate per 8-row block
                nblk = R // G
                s1 = spool.tile([BL, R // 2, KD], F32, tag="s1")
                pairs = kc[:, :, :].rearrange("p (a two) d -> p a two d", two=2)
                nc.vector.tensor_tensor(
                    out=s1[:, :, :],
                    in0=pairs[:, :, 0, :],
                    in1=pairs[:, :, 1, :],
                    op=add,
                )
                r = R // 2
                while r > nblk:
                    r = _inplace_fold(nc.vector, s1, r, add)
                for j in range(nblk):
                    if seed:
                        nc.vector.tensor_scalar(
                            out=acc[:, :],
                            in0=s1[:, j, :],
                            scalar1=bm_t[:, jg + j : jg + j + 1],
                            scalar2=None,
                            op0=mult,
                        )
                        seed = False
                    else:
                        nc.vector.scalar_tensor_tensor(
                            out=acc[:, :],
                            in0=s1[:, j, :],
                            scalar=bm_t[:, jg + j : jg + j + 1],
                            in1=acc[:, :],
                            op0=mult,
                            op1=add,
                        )

            # chunk 0 (tiny) seeds acc as soon as its DMA lands
            chunk_chain(kcs[0][0], kcs[0][1], 0, seed=True)

            # correction chain into acc2 fills the DMA ramp-up window
            acc2 = singles.tile([BL, KD], F32)
            for t in range(G):
                if t == 0:
                    nc.vector.tensor_scalar(
                        out=acc2[:, :],
                        in0=kg_t[:, t, :],
                        scalar1=sm_t[:, t : t + 1],
                        scalar2=None,
                        op0=mult,
                    )
                else:
                    nc.vector.scalar_tensor_tensor(
                        out=acc2[:, :],
                        in0=kg_t[:, t, :],
                        scalar=sm_t[:, t : t + 1],
                        in1=acc2[:, :],
                        op0=mult,
                        op1=add,
                    )

            # qsum (single contiguous-innermost reduce) + acc2 merge, still
            # inside the ramp window
            qs = singles.tile([BL, D], F32)
            nc.vector.reduce_sum(out=qs[:, :], in_=q_t[:, :, :], axis=AX)
            nc.vector.tensor_add(out=acc[:, :], in0=acc[:, :], in1=acc2[:, :])

            # bulk chunks
            jg = kcs[0][1] // G
            for kc, R in kcs[1:]:
                chunk_chain(kc, R, jg, seed=False)
                jg += R // G
            ksum = acc

            # --- mix (3x3 softmaxed weights), scale by qsum, softmax over D ---
            obuf = singles.tile([BL, KD], F32)
            for r3 in range(3):
                t1 = small.tile([BL, D], F32, tag="t1")
                nc.scalar.mul(
                    out=t1[:, :],
                    in_=ksum[:, 2 * D : 3 * D],
                    mul=w_t[:, 3 * r3 + 2 : 3 * r3 + 3],
                )
                t2 = small.tile([BL, D], F32, tag="t2")
                nc.vector.scalar_tensor_tensor(
                    out=t2[:, :],
                    in0=ksum[:, D : 2 * D],
                    scalar=w_t[:, 3 * r3 + 1 : 3 * r3 + 2],
                    in1=t1[:, :],
                    op0=mult,
                    op1=add,
                )
                t3 = small.tile([BL, D], F32, tag="t3")
                nc.vector.scalar_tensor_tensor(
                    out=t3[:, :],
                    in0=ksum[:, 0:D],
                    scalar=w_t[:, 3 * r3 : 3 * r3 + 1],
                    in1=t2[:, :],
                    op0=mult,
                    op1=add,
                )
                s_r = small.tile([BL, D], F32, tag="sr")
                nc.vector.tensor_mul(out=s_r[:, :], in0=t3[:, :], in1=qs[:, :])
                mx = small.tile([BL, 1], F32, tag="mx")
                nc.vector.reduce_max(out=mx[:, :], in_=s_r[:, :], axis=AX)
                nmx = small.tile([BL, 1], F32, tag="nmx")
                nc.scalar.mul(out=nmx[:, :], in_=mx[:, :], mul=-1.0)
                ex = small.tile([BL, D], F32, tag="ex")
                esum = small.tile([BL, 1], F32, tag="esum")
                nc.scalar.activation(
                    out=ex[:, :],
                    in_=s_r[:, :],
                    func=mybir.ActivationFunctionType.Exp,
                    bias=nmx[:, :],
                    scale=1.0,
                    accum_out=esum[:, :],
                )
                rec = small.tile([BL, 1], F32, tag="rec")
                nc.vector.reciprocal(out=rec[:, :], in_=esum[:, :])
                nc.scalar.activation(
                    out=obuf[:, r3 * D : (r3 + 1) * D],
                    in_=ex[:, :],
                    func=mybir.ActivationFunctionType.Copy,
                    bias=0.0,
                    scale=rec[:, :],
                )

            nc.sync.dma_start(out=out_d, in_=obuf[:, :])

    nc.compile()
    return nc


def _get_module():
    nc = _CACHE.get("nc")
    if nc is None:
        nc = _build_module()
        _CACHE["nc"] = nc
    return nc


def _prepare_in_maps(q, k, kes, W):
    lens = kes.reshape(B).astype(np.int64)
    j0 = lens // G
    rem = lens % G
    rows = (j0[:, None] * G + np.arange(G)[None, :]).clip(0, LK - 1)  # (B, G)
    kg = k[np.arange(B)[:, None], rows, :]  # (B, G, KD)
    bm = ((np.arange(NB)[None, :] + 1) * G <= lens[:, None]).astype(np.float32)
    sm = (np.arange(G)[None, :] < rem[:, None]).astype(np.float32)
    w_rep = np.tile(W.reshape(1, 9), (B, 1)).astype(np.float32)
    meta = np.concatenate([w_rep, bm, sm], axis=1).astype(np.float32)  # (B, 42)

    in_maps = []
    for c in range(NCORES):
        s = slice(c * BL, (c + 1) * BL)
        in_maps.append(
            {
                "q": np.ascontiguousarray(q[s].transpose(0, 2, 1)),
                "k": np.ascontiguousarray(k[s]),
                "kg": np.ascontiguousarray(kg[s]),
                "meta": np.ascontiguousarray(meta[s]),
            }
        )
    return in_maps


def _run(q, k, kes_length, mss_weight, **run_kwargs):
    q = np.ascontiguousarray(np.asarray(q, dtype=np.float32))
    k = np.ascontiguousarray(np.asarray(k, dtype=np.float32))
    kes = np.asarray(kes_length).astype(np.int32)
    m = np.asarray(mss_weight, dtype=np.float32)
    e = np.exp(m - m.max(axis=1, keepdims=True))
    W = (e / e.sum(axis=1, keepdims=True)).astype(np.float32)

    nc = _get_module()
    in_maps = _prepare_in_maps(q, k, kes, W)
    res = run_bass_kernel_spmd(nc, in_maps, core_ids=list(range(NCORES)), **run_kwargs)
    out = np.concatenate([res.results[c]["out"] for c in range(NCORES)], axis=0)
    return out.reshape(B, 1, KD).astype(np.float32), res


def kernel(q, k, v=None, kes_length=None, mss_weight=None, **_):
    out, _res = _run(q, k, kes_length, mss_weight)
    return out
